# revision 1
# baseline (speedup 1.0000x reference)
"""AdaBIGGAN adaptive 1x1-conv stage, data-parallel across 8 TRN2 NeuronCores.

Math (per sample b):
    scale[b, c] = sum_k y[b, k] * Wsum[c, k] + bsum[c]
        where Wsum[c, k] = sum_j Wg_w[c*C + j, k],  bsum[c] = sum_j Wg_b[c*C + j]
    bias[b, c]  = sum_k y[b, k] * Bg_w[c, k] + Bg_b[c]
    out[b, c, :, :] = relu(h[b, c, :, :] * scale[b, c] + bias[b, c])

Sharding: batch B=32 split 4-per-core across 8 cores; hypernet params replicated.

Wg_w/Wg_b keep their natural [c, (j ...)] layout but are host-padded to 128
partitions with replicas of rows c = 0..31, so (a) the loads run at full
16-port DMA bandwidth and (b) after the j-fold the first stream row-tile's
scale factors are already in the right partitions (flat index b*C+c: rows
96..127 of tile 0 are exactly c = 0..31 of batch 1). Row tiles 1/2 need only
two partition-shift SBUF->SBUF copies each. The per-sample dot products run
directly in the flat layout against host-prepared y / [Bg_w | Bg_b] tables
(ones-augmented to fold in the additive constants).
"""

import numpy as np

import concourse.bacc as bacc
import concourse.mybir as mybir
from concourse.tile import TileContext
from concourse.bass_utils import run_bass_kernel_spmd

_B, _C, _H, _W, _IN = 32, 96, 128, 128, 148
_NCORES = 8
_BL = _B // _NCORES          # 4 samples per core
_HW = _H * _W                # 16384
_ROWS = _BL * _C             # 384 rows = 3 x 128 partitions
_NPT = 3                     # row tiles of 128
_FCH = 4096                  # free-dim chunk of the h stream
_WGC = 8                     # Wg_w load chunks (along j)
_JC = _C // _WGC             # 12 j rows per chunk
_CKA = 111                   # k-cols per partition in the balanced Wg_w layout
_F32 = mybir.dt.float32

LAST_RESULTS = None


def _segments(r):
    """Flat rows [128r, 128r+128) split at batch boundaries -> (p0, c0, n)."""
    segs = []
    p = 0
    while p < 128:
        f = r * 128 + p
        c = f % _C
        n = min(128 - p, _C - c)
        segs.append((p, c, n))
        p += n
    return segs


def _build():
    nc = bacc.Bacc(None)
    h = nc.declare_dram_parameter("h", [_ROWS, _HW], _F32, isOutput=False)
    wgh = nc.declare_dram_parameter("wgh", [128, _C * _CKA], _F32, isOutput=False)
    yaf = nc.declare_dram_parameter("yaf", [_ROWS, _IN + 1], _F32, isOutput=False)
    bwf = nc.declare_dram_parameter("bwf", [_ROWS, _IN + 1], _F32, isOutput=False)
    wb = nc.declare_dram_parameter("wb", [_C, _C], _F32, isOutput=False)
    out = nc.declare_dram_parameter("out", [_ROWS, _HW], _F32, isOutput=True)

    with TileContext(nc) as tc:
        with (
            tc.tile_pool(name="hyper", bufs=1) as hp,
            tc.tile_pool(name="stream", bufs=8) as sp,
        ):
            # --- hypernet loads (both HWDGE rings, ahead of the h stream) ----
            chunks = []
            for m in range(_WGC):
                wg_m = hp.tile([128, _JC * _CKA], _F32, tag=f"wg{m}")
                eng = nc.sync if m % 2 == 0 else nc.scalar
                eng.dma_start(
                    out=wg_m[:], in_=wgh[:, m * _JC * _CKA:(m + 1) * _JC * _CKA])
                chunks.append(wg_m)
            ya_t, bw_t = [], []
            for r in range(_NPT):
                yt = hp.tile([128, _IN + 1], _F32, tag=f"ya{r}")
                nc.gpsimd.dma_start(out=yt[:], in_=yaf[r * 128:(r + 1) * 128, :])
                ya_t.append(yt)
                bt = hp.tile([128, _IN + 1], _F32, tag=f"bw{r}")
                nc.gpsimd.dma_start(out=bt[:], in_=bwf[r * 128:(r + 1) * 128, :])
                bw_t.append(bt)
            wb_t = hp.tile([_C, _C], _F32)         # [c, j]
            nc.gpsimd.dma_start(out=wb_t[:], in_=wb[:])

            # --- bias side: zero device-compute dependencies -----------------
            bias_fl, scale_fl = [], [None] * _NPT
            jb = hp.tile([128, _IN + 1], _F32)
            for r in range(_NPT):
                bf = hp.tile([128, 1], _F32, tag=f"bf{r}")
                nc.vector.scalar_tensor_tensor(
                    out=jb[:], in0=bw_t[r][:], scalar=1.0, in1=ya_t[r][:],
                    op0=mybir.AluOpType.mult, op1=mybir.AluOpType.mult,
                    accum_out=bf[:],
                )
                bias_fl.append(bf)

            # --- fold j: accumulate chunks, halve 12 -> 6 -> 3, reduce -------
            acc = chunks[0]                        # [128, (12 j, 148 k)]
            for m in range(1, _WGC):
                nc.vector.tensor_add(acc[:], acc[:], chunks[m][:])
            nc.vector.tensor_add(acc[:, :6 * _CKA], acc[:, :6 * _CKA],
                                 acc[:, 6 * _CKA:12 * _CKA])
            nc.vector.tensor_add(acc[:, :3 * _CKA], acc[:, :3 * _CKA],
                                 acc[:, 3 * _CKA:6 * _CKA])
            wflat = hp.tile([128, _CKA], _F32)
            nc.vector.tensor_reduce(
                out=wflat[:],
                in_=acc[:, :3 * _CKA].rearrange("p (j l) -> p l j", j=3, l=_CKA),
                axis=mybir.AxisListType.X,
                op=mybir.AluOpType.add,
            )
            # assemble Wsum [c, 148]: partitions 0-95 hold k<111; partitions
            # 96+q hold k>=111 for channels {q, q+32, q+64}
            wsum = hp.tile([_C, _IN], _F32)
            nc.gpsimd.dma_start(out=wsum[:, 0:_CKA], in_=wflat[0:_C, :])
            for i in range(3):
                nc.gpsimd.dma_start(
                    out=wsum[32 * i:32 * (i + 1), _CKA:_IN],
                    in_=wflat[_C:128, i * 37:(i + 1) * 37])
            bsum = hp.tile([_C, 1], _F32)
            nc.vector.tensor_reduce(
                out=bsum[:], in_=wb_t[:],
                axis=mybir.AxisListType.X, op=mybir.AluOpType.add,
            )

            # scale dots per row tile, directly in the flat layout. Tile 0 is
            # already aligned; tiles 1/2 take two partition-shift copies.
            js = hp.tile([128, _IN], _F32)
            for r in range(_NPT):
                wsr = hp.tile([128, _IN], _F32, tag=f"ws{r}")
                bsr = hp.tile([128, 1], _F32, tag=f"bs{r}")
                for (p0, c0, n) in _segments(r):
                    nc.gpsimd.dma_start(out=wsr[p0:p0 + n, :],
                                        in_=wsum[c0:c0 + n, :])
                    nc.gpsimd.dma_start(out=bsr[p0:p0 + n, :],
                                        in_=bsum[c0:c0 + n, :])
                sf = hp.tile([128, 1], _F32, tag=f"sf{r}")
                nc.vector.scalar_tensor_tensor(
                    out=js[:], in0=wsr[:], scalar=1.0, in1=ya_t[r][:, :_IN],
                    op0=mybir.AluOpType.mult, op1=mybir.AluOpType.mult,
                    accum_out=sf[:],
                )
                nc.vector.tensor_add(sf[:], sf[:], bsr[:])
                scale_fl[r] = sf

            # --- stream h: out = relu(h * scale + bias), fused in ScalarE ----
            # loads on sync HWDGE ring, stores on scalar HWDGE ring. The last
            # row-tile's final chunk is split fine-grained so the store tail
            # drains right behind the last loads instead of lagging 2 chunks.
            plan = []
            for r in range(_NPT):
                f0 = 0
                while f0 < _HW:
                    if r == _NPT - 1 and f0 == _HW - _FCH:
                        for w in (2048, 1024, 512, 512):
                            plan.append((r, f0, w))
                            f0 += w
                    else:
                        plan.append((r, f0, _FCH))
                        f0 += _FCH
            n_chunks = len(plan)
            for ci, (r, f0, w) in enumerate(plan):
                rows = slice(r * 128, (r + 1) * 128)
                t = sp.tile([128, _FCH], _F32, tag="st")
                # early loads also ride the (still store-free) scalar ring;
                # the final stores also ride the (by then load-free) sync
                # ring, so the drain uses both rings
                ld = nc.scalar if ci in (1, 3, 5) else nc.sync
                ld.dma_start(out=t[:, :w], in_=h[rows, f0:f0 + w])
                nc.scalar.activation(
                    out=t[:, :w], in_=t[:, :w],
                    func=mybir.ActivationFunctionType.Relu,
                    bias=bias_fl[r][:],
                    scale=scale_fl[r][:],
                )
                st = nc.sync if ci >= n_chunks - 5 else nc.scalar
                st.dma_start(out=out[rows, f0:f0 + w], in_=t[:, :w])
    nc.finalize()
    return nc


def kernel(h, y, Wg_w, Wg_b, Bg_w, Bg_b):
    global LAST_RESULTS
    h = np.ascontiguousarray(np.asarray(h), np.float32)
    y = np.ascontiguousarray(np.asarray(y), np.float32)
    Wg_w = np.ascontiguousarray(np.asarray(Wg_w), np.float32)
    Wg_b = np.ascontiguousarray(np.asarray(Wg_b), np.float32)
    Bg_w = np.ascontiguousarray(np.asarray(Bg_w), np.float32)
    Bg_b = np.ascontiguousarray(np.asarray(Bg_b), np.float32)

    nc = _build()
    w3 = Wg_w.reshape(_C, _C, _IN)                         # [c, j, k]
    part_a = w3[:, :, :_CKA].reshape(_C, _C * _CKA)        # [c, (j k<111)]
    part_b = (w3[:, :, _CKA:].reshape(3, 32, _C, 37)       # [i, q, j, k37]
              .transpose(1, 2, 0, 3).reshape(32, _C * _CKA))
    wgh_r = np.ascontiguousarray(np.vstack([part_a, part_b]))
    wb_r = np.ascontiguousarray(Wg_b.reshape(_C, _C))
    # [Bg_w | Bg_b] rows tiled to the flat [b*C + c] layout
    bw_aug = np.concatenate([Bg_w, Bg_b.reshape(_C, 1)], 1)
    bwf_r = np.ascontiguousarray(np.tile(bw_aug, (_BL, 1)))

    in_maps = []
    for i in range(_NCORES):
        hs = h[i * _BL:(i + 1) * _BL].reshape(_ROWS, _HW)
        ys = y[i * _BL:(i + 1) * _BL]          # [4, 148]
        y_aug = np.concatenate([ys, np.ones((_BL, 1), np.float32)], 1)
        in_maps.append({
            "h": np.ascontiguousarray(hs),
            "wgh": wgh_r,
            "yaf": np.ascontiguousarray(np.repeat(y_aug, _C, axis=0)),
            "bwf": bwf_r,
            "wb": wb_r,
        })

    res = run_bass_kernel_spmd(nc, in_maps, core_ids=list(range(_NCORES)))
    LAST_RESULTS = res
    outs = [r["out"].reshape(_BL, _C, _H, _W) for r in res.results]
    return np.concatenate(outs, axis=0)

